# revision 15
# baseline (speedup 1.0000x reference)
"""Trainium2 Bass kernel for nn_CACSegmentor (segment_reduce).

Strategy: shard N=524288 points across 8 cores (65536 each; each core's
slice lies in one point cloud b=core//2). Two SPMD launches:
  pass A: per-point seg logits + softmax P; one fused PE matmul accumulates
          [M=x^T x | S1=P^T x | segment sums | counts | Sigma x]; CE(seg)
          partial sums; stores transposed bf16 feat for pass B.
  host:   tiny [K,C] glue (BN stats from M, proto mlps, weight folds).
  pass B: h=W1 x -> relu_b/relu_f -> z=G relu (norm via quadratic form),
          rl/cac cosine logits, softmax losses, per-class sums via OH matmul.
"""
import sys, os
sys.path.insert(0, "/opt/trn_rl_repo")

import numpy as np
import ml_dtypes
from contextlib import ExitStack

import concourse.bass as bass
import concourse.bacc as bacc
import concourse.tile as tile
from concourse import mybir
from concourse import bass_utils
from concourse.ap import AP

N, C, K, B, NCORES = 524288, 96, 20, 4, 8
NPC = N // NCORES
COS = 15.0
BF = mybir.dt.bfloat16
F32 = mybir.dt.float32
I32 = mybir.dt.int32
bfnp = ml_dtypes.bfloat16
AF = mybir.ActivationFunctionType
OP = mybir.AluOpType
AX = mybir.AxisListType

_CACHE = {}


def _default_runner(nc, in_maps):
    res = bass_utils.run_bass_kernel_spmd(nc, in_maps, list(range(len(in_maps))))
    return res.results


_RUNNER = _default_runner


def _bc(ap, axis, n):
    """Insert a broadcast (0-stride) dim of size n at position axis."""
    return ap.unsqueeze(axis).broadcast_to(
        tuple(ap.shape[:axis]) + (n,) + tuple(ap.shape[axis:]))


# ---------------------------------------------------------------- pass A ----
def _build_passA(npc):
    T = 512
    NMT = npc // T
    nc = bacc.Bacc("TRN2", target_bir_lowering=False, debug=False)
    feat = nc.dram_tensor("feat", [npc, C], F32, kind="ExternalInput").ap()
    tga = nc.dram_tensor("tga", [NMT, 128, 4], I32, kind="ExternalInput").ap()
    ident = nc.dram_tensor("ident", [128, 128], BF, kind="ExternalInput").ap()
    segwb = nc.dram_tensor("segwb", [C + 1, K], BF, kind="ExternalInput").ap()
    kidxrow = nc.dram_tensor("kidxrow", [1, 4 * K], I32, kind="ExternalInput").ap()
    xstore = nc.dram_tensor("xstore", [C, npc], BF, kind="ExternalOutput").ap()
    outM = nc.dram_tensor("outM", [C + 1, C + 1 + 2 * K], F32, kind="ExternalOutput").ap()
    outnll = nc.dram_tensor("outnll", [128, 2], F32, kind="ExternalOutput").ap()

    with tile.TileContext(nc) as tc, ExitStack() as ctx:
        const = ctx.enter_context(tc.tile_pool(name="const", bufs=1))
        identt = const.tile([128, 128], BF)
        nc.sync.dma_start(identt[:], ident)
        segwt = const.tile([C + 1, K], BF)
        nc.sync.dma_start(segwt[:], segwb)
        kid = const.tile([1, 4 * K], I32)
        nc.sync.dma_start(kid[:], kidxrow)
        kidx4 = const.tile([128, 4 * K], I32)
        nc.gpsimd.partition_broadcast(kidx4[:], kid[:])

        acc = ctx.enter_context(tc.tile_pool(name="acc", bufs=1))
        sBb = acc.tile([128, NMT * 4], F32)
        vfb = acc.tile([128, NMT * 4], F32)
        acc2b = acc.tile([128, NMT], F32)
        scrap = acc.tile([128, 4 * K], BF)
        scrap2 = acc.tile([128, NMT * 4], F32)

        psA = ctx.enter_context(tc.tile_pool(name="psA", bufs=2, space="PSUM"))
        psM = ctx.enter_context(tc.tile_pool(name="psM", bufs=1, space="PSUM"))
        bigM = psM.tile([C + 1, C + 1 + 2 * K], F32)

        sb = ctx.enter_context(tc.tile_pool(name="sbA", bufs=3))

        for m in range(NMT):
            xf = sb.tile([128, 4, C], F32, tag="xf")
            for a in range(4):
                nc.sync.dma_start(
                    xf[:, a, :], feat[m * 512 + a * 128: m * 512 + (a + 1) * 128, :])
            tg = sb.tile([128, 4], I32, tag="tg")
            nc.sync.dma_start(tg[:], tga[m])

            # xe free layout: [0:C]=x bf16, [C]=ones, [C+1:C+1+K]=P, [C+1+K:C+1+2K]=OH
            xe = sb.tile([128, 4, C + 1 + 2 * K], BF, tag="xe")
            nc.vector.tensor_copy(xe[:, :, 0:C], xf[:])
            nc.vector.memset(xe[:, :, C:C + 1], 1.0)

            xtp = psA.tile([C + 1, 512], BF, tag="xtp")
            for a in range(4):
                nc.tensor.transpose(
                    xtp[:, a * 128:(a + 1) * 128], xe[:, a, 0:C + 1], identt[:])
            xts = sb.tile([C + 1, 512], BF, tag="xts")
            nc.vector.tensor_copy(xts[:], xtp[:])
            nc.sync.dma_start(xstore[:, m * 512:(m + 1) * 512], xts[0:C, :])

            segp = psA.tile([128, 4, K], F32, tag="segp")
            for a in range(4):
                nc.tensor.matmul(
                    segp[:, a, :], xts[:, a * 128:(a + 1) * 128], segwt[:],
                    start=True, stop=True)

            esb = sb.tile([128, 4, K], F32, tag="esb")
            nc.scalar.activation(esb[:], segp[:], AF.Exp)
            nc.vector.tensor_reduce(
                sBb[:, m * 4:(m + 1) * 4], esb[:], axis=AX.X, op=OP.add)
            rec = sb.tile([128, 4], F32, tag="rec")
            nc.vector.reciprocal(rec[:], sBb[:, m * 4:(m + 1) * 4])
            nc.vector.tensor_tensor(
                xe[:, :, C + 1:C + 1 + K], esb[:], _bc(rec[:], 2, K), op=OP.mult)

            oh = xe[:, :, C + 1 + K:C + 1 + 2 * K]
            nc.vector.tensor_tensor(
                oh, kidx4[:].rearrange("p (a k) -> p a k", a=4),
                _bc(tg[:], 2, K), op=OP.is_equal)
            nc.vector.tensor_reduce(
                vfb[:, m * 4:(m + 1) * 4], oh, axis=AX.X, op=OP.add)
            nc.vector.scalar_tensor_tensor(
                scrap[:].rearrange("p (a k) -> p a k", a=4), oh, 1.0, segp[:],
                op0=OP.mult, op1=OP.mult, accum_out=acc2b[:, m:m + 1])

            for a in range(4):
                nc.tensor.matmul(
                    bigM[:], xe[:, a, 0:C + 1], xe[:, a, 0:C + 1 + 2 * K],
                    start=(m == 0 and a == 0), stop=(m == NMT - 1 and a == 3))

        lnb = acc.tile([128, NMT * 4], F32)
        nc.scalar.activation(lnb[:], sBb[:], AF.Ln)
        accVL = acc.tile([128, 1], F32)
        nc.vector.tensor_tensor(scrap2[:], vfb[:], lnb[:], op=OP.mult)
        nc.vector.tensor_reduce(accVL[:], scrap2[:], axis=AX.X, op=OP.add)
        acc2r = acc.tile([128, 1], F32)
        nc.vector.tensor_reduce(acc2r[:], acc2b[:], axis=AX.X, op=OP.add)
        nc.sync.dma_start(outnll[:, 0:1], accVL[:])
        nc.sync.dma_start(outnll[:, 1:2], acc2r[:])
        bigMs = acc.tile([C + 1, C + 1 + 2 * K], F32)
        nc.vector.tensor_copy(bigMs[:], bigM[:])
        nc.sync.dma_start(outM[:], bigMs[:])

    nc.compile()
    return nc


# ---------------------------------------------------------------- pass B ----
def _build_passB(npc, has_c0, has_v, has_cb):
    T = 512
    NMT = npc // T
    nc = bacc.Bacc("TRN2", target_bir_lowering=False, debug=False)
    xst = nc.dram_tensor("xst", [C, npc], BF, kind="ExternalInput").ap()
    tgb = nc.dram_tensor("tgb", [NMT, 128, 4], I32, kind="ExternalInput").ap()
    w1t = nc.dram_tensor("w1t", [C, C], BF, kind="ExternalInput").ap()
    gbt = nc.dram_tensor("gbt", [C, C], BF, kind="ExternalInput").ap()
    gft = nc.dram_tensor("gft", [C, C], BF, kind="ExternalInput").ap()
    wrlt = nc.dram_tensor("wrlt", [C, K], BF, kind="ExternalInput").ap()
    wcact = nc.dram_tensor("wcact", [C, K], BF, kind="ExternalInput").ap()
    tbv = nc.dram_tensor("tbv", [C, 1], F32, kind="ExternalInput").ap()
    tfv = nc.dram_tensor("tfv", [C, 1], F32, kind="ExternalInput").ap()
    vbv = nc.dram_tensor("vbv", [C, 1], F32, kind="ExternalInput").ap()
    vfv = nc.dram_tensor("vfv", [C, 1], F32, kind="ExternalInput").ap()
    ones96 = nc.dram_tensor("ones96", [C, 1], BF, kind="ExternalInput").ap()
    ident = nc.dram_tensor("ident", [128, 128], BF, kind="ExternalInput").ap()
    kidxrow = nc.dram_tensor("kidxrow", [1, 4 * K], I32, kind="ExternalInput").ap()
    cbrow = nc.dram_tensor("cbrow", [1, 2 * K], F32, kind="ExternalInput").ap()
    c0row = nc.dram_tensor("c0row", [1, 2], F32, kind="ExternalInput").ap()
    outcols = nc.dram_tensor("outcols", [4, K], F32, kind="ExternalOutput").ap()

    LN15 = float(np.log(COS))

    with tile.TileContext(nc) as tc, ExitStack() as ctx:
        const = ctx.enter_context(tc.tile_pool(name="const", bufs=1))

        def cload(apdram, shape, dt):
            t = const.tile(shape, dt, tag=f"c_{apdram.name}")
            nc.sync.dma_start(t[:], apdram)
            return t

        w1tt = cload(w1t, [C, C], BF)
        gbtt = cload(gbt, [C, C], BF)
        gftt = cload(gft, [C, C], BF)
        wrltt = cload(wrlt, [C, K], BF)
        wcactt = cload(wcact, [C, K], BF)
        tbt = cload(tbv, [C, 1], F32)
        tft = cload(tfv, [C, 1], F32)
        vbt = cload(vbv, [C, 1], F32)
        vft = cload(vfv, [C, 1], F32)
        onest = cload(ones96, [C, 1], BF)
        identt = cload(ident, [128, 128], BF)
        kid = cload(kidxrow, [1, 4 * K], I32)
        kidx4 = const.tile([128, 4 * K], I32)
        nc.gpsimd.partition_broadcast(kidx4[:], kid[:])
        cbr = cload(cbrow, [1, 2 * K], F32)
        cbbc = const.tile([128, 2 * K], F32)
        nc.gpsimd.partition_broadcast(cbbc[:], cbr[:])
        c0r = cload(c0row, [1, 2], F32)
        c0bc = const.tile([128, 2], F32)
        nc.gpsimd.partition_broadcast(c0bc[:], c0r[:])
        bias15 = const.tile([128, 1], F32)
        nc.vector.memset(bias15[:], LN15)
        bias4 = const.tile([128, 1], F32)
        nc.vector.memset(bias4[:], 1e-4)

        psB = ctx.enter_context(tc.tile_pool(name="psB", bufs=2, space="PSUM"))
        psU = ctx.enter_context(tc.tile_pool(name="psU", bufs=1, space="PSUM"))
        psC = ctx.enter_context(tc.tile_pool(name="psC", bufs=1, space="PSUM"))
        colacc = psC.tile([4, K], F32)
        sb = ctx.enter_context(tc.tile_pool(name="sbB", bufs=3))

        for m in range(NMT):
            xt = sb.tile([C, 512], BF, tag="xt")
            nc.sync.dma_start(xt[:], xst[:, m * 512:(m + 1) * 512])
            tg = sb.tile([128, 4], I32, tag="tg")
            nc.sync.dma_start(tg[:], tgb[m])

            hp = psB.tile([C, 512], F32, tag="hp")
            nc.tensor.matmul(hp[:], w1tt[:], xt[:], start=True, stop=True)
            rb = sb.tile([C, 512], BF, tag="rb")
            nc.scalar.activation(rb[:], hp[:], AF.Relu, bias=tbt[:])
            rf = sb.tile([C, 512], BF, tag="rf")
            nc.vector.tensor_scalar(
                rf[:], hp[:], tft[:], 0.0, op0=OP.add, op1=OP.max)

            zb = psB.tile([C, 512], F32, tag="z")
            nc.tensor.matmul(zb[:], gbtt[:], rb[:], start=True, stop=True)
            pb = sb.tile([C, 512], BF, tag="pb")
            if has_v:
                nc.vector.scalar_tensor_tensor(
                    pb[:], zb[:], vbt[:], rb[:], op0=OP.add, op1=OP.mult)
            else:
                nc.vector.tensor_tensor(pb[:], zb[:], rb[:], op=OP.mult)
            zf = psB.tile([C, 512], F32, tag="z")
            nc.tensor.matmul(zf[:], gftt[:], rf[:], start=True, stop=True)
            pf = sb.tile([C, 512], BF, tag="pf")
            if has_v:
                nc.vector.scalar_tensor_tensor(
                    pf[:], zf[:], vft[:], rf[:], op0=OP.add, op1=OP.mult)
            else:
                nc.vector.tensor_tensor(pf[:], zf[:], rf[:], op=OP.mult)

            # transpose p_b/p_f subtiles to [pts, C] and reduce -> per-point norms
            s2p = sb.tile([128, 4, 2], F32, tag="s2p")
            for pi, pt in enumerate((pb, pf)):
                ptt = psU.tile([128, 4, C], BF, tag="ptt")
                for a in range(4):
                    nc.tensor.transpose(
                        ptt[:, a, :], pt[:, a * 128:(a + 1) * 128],
                        identt[0:C, 0:C])
                nc.vector.tensor_reduce(
                    s2p[:, :, pi], ptt[:], axis=AX.X, op=OP.add)
            # clamp (and +c0): nsq' = max(nsq + c0, 1e-24)
            if has_c0:
                nc.vector.tensor_tensor(
                    s2p[:], s2p[:], _bc(c0bc[:], 1, 4), op=OP.add)
            nc.vector.tensor_scalar(
                s2p[:], s2p[:], 1e-24, None, op0=OP.max)
            lnn = sb.tile([128, 4, 2], F32, tag="lnn")
            nc.scalar.activation(lnn[:], s2p[:], AF.Ln)
            st = sb.tile([128, 4, 2], F32, tag="st")
            nc.scalar.activation(st[:], lnn[:], AF.Exp, scale=-0.5, bias=bias15[:])

            up = psU.tile([128, 4, 2, K], F32, tag="up")
            for a in range(4):
                nc.tensor.matmul(
                    up[:, a, 0, :], rb[:, a * 128:(a + 1) * 128], wrltt[:],
                    start=True, stop=True)
                nc.tensor.matmul(
                    up[:, a, 1, :], rf[:, a * 128:(a + 1) * 128], wcactt[:],
                    start=True, stop=True)

            rl = sb.tile([128, 4, 2, K], F32, tag="rl")
            if has_cb:
                nc.vector.tensor_tensor(
                    rl[:], up[:],
                    _bc(cbbc[:].rearrange("p (t k) -> p t k", t=2), 1, 4),
                    op=OP.add)
                nc.vector.tensor_tensor(rl[:], rl[:], _bc(st[:], 3, K), op=OP.mult)
            else:
                nc.vector.tensor_tensor(rl[:], up[:], _bc(st[:], 3, K), op=OP.mult)

            e = sb.tile([128, 4, 2, K], F32, tag="e")
            nc.scalar.activation(e[:], rl[:], AF.Exp)
            se = sb.tile([128, 4, 2], F32, tag="se")
            nc.vector.tensor_reduce(se[:], e[:], axis=AX.X, op=OP.add)
            lnse = sb.tile([128, 4, 2], F32, tag="lnse")
            nc.scalar.activation(lnse[:], se[:], AF.Ln)
            rse = sb.tile([128, 4], F32, tag="rse")
            nc.vector.reciprocal(rse[:], se[:, :, 1])

            sm = sb.tile([128, 4, K], F32, tag="sm")
            nc.vector.tensor_tensor(sm[:], e[:, :, 1, :], _bc(rse[:], 2, K),
                                    op=OP.mult)
            lsm0 = sb.tile([128, 4, K], F32, tag="lsm0")
            nc.scalar.activation(lsm0[:], sm[:], AF.Ln, bias=bias4[:])

            oh = sb.tile([128, 4, K], BF, tag="oh")
            nc.vector.tensor_tensor(
                oh[:], kidx4[:].rearrange("p (a k) -> p a k", a=4),
                _bc(tg[:], 2, K), op=OP.is_equal)

            cols = sb.tile([128, 4, 4], F32, tag="cols")
            tmp = sb.tile([128, 4, K], F32, tag="tmp")
            # ent' = sum sm*ln(sm+1e-4)  -> cols[:,:,1]
            nc.vector.tensor_tensor(tmp[:], sm[:], lsm0[:], op=OP.mult)
            nc.vector.tensor_reduce(cols[:, :, 1], tmp[:], axis=AX.X, op=OP.add)
            # lsm_rl = rl_b - lnse_b
            lsmrl = sb.tile([128, 4, K], F32, tag="lsmrl")
            nc.vector.tensor_tensor(
                lsmrl[:], rl[:, :, 0, :], _bc(lnse[:, :, 0], 2, K), op=OP.subtract)
            # A = sum lsm_rl * e_cac
            At = sb.tile([128, 4], F32, tag="At")
            nc.vector.tensor_tensor(tmp[:], lsmrl[:], e[:, :, 1, :], op=OP.mult)
            nc.vector.tensor_reduce(At[:], tmp[:], axis=AX.X, op=OP.add)
            # Bv = sum lsm_rl * OH -> cols[:,:,2]
            nc.vector.tensor_tensor(tmp[:], lsmrl[:], oh[:], op=OP.mult)
            nc.vector.tensor_reduce(cols[:, :, 2], tmp[:], axis=AX.X, op=OP.add)
            # nllc = sum (cac - lnse_cac) * OH -> cols[:,:,3]
            lsmc = sb.tile([128, 4, K], F32, tag="lsmc")
            nc.vector.tensor_tensor(
                lsmc[:], rl[:, :, 1, :], _bc(lnse[:, :, 1], 2, K), op=OP.subtract)
            nc.vector.tensor_tensor(tmp[:], lsmc[:], oh[:], op=OP.mult)
            nc.vector.tensor_reduce(cols[:, :, 3], tmp[:], axis=AX.X, op=OP.add)
            # le'' = (A*rse + Bv) * ent' -> cols[:,:,0]
            lp = sb.tile([128, 4], F32, tag="lp")
            nc.vector.tensor_tensor(lp[:], At[:], rse[:], op=OP.mult)
            nc.vector.tensor_tensor(lp[:], lp[:], cols[:, :, 2], op=OP.add)
            nc.vector.tensor_tensor(cols[:, :, 0], lp[:], cols[:, :, 1], op=OP.mult)

            colsb = sb.tile([128, 4, 4], BF, tag="colsb")
            nc.vector.tensor_copy(colsb[:], cols[:])
            for a in range(4):
                nc.tensor.matmul(
                    colacc[:], colsb[:, a, :], oh[:, a, :],
                    start=(m == 0 and a == 0), stop=(m == NMT - 1 and a == 3))

        colsout = const.tile([4, K], F32)
        nc.vector.tensor_copy(colsout[:], colacc[:])
        nc.sync.dma_start(outcols[:], colsout[:])

    nc.compile()
    return nc


# ------------------------------------------------------------- host glue ----
def _l2n(x, axis=1):
    n = np.linalg.norm(x, axis=axis, keepdims=True)
    return x / np.maximum(n, 1e-12)


def _mlp2(x, w1, w2, b2):
    return np.maximum(x @ w1.T, 0.0) @ w2.T + b2


def _prep_common(target_i32, npc):
    NMT = npc // 512
    tg = target_i32.reshape(NCORES, NMT, 4, 128).transpose(0, 1, 3, 2)
    return np.ascontiguousarray(tg)  # [NCORES, NMT, 128, 4]


def kernel(**inputs):
    feat = np.asarray(inputs["feat"], np.float32)
    target = np.asarray(inputs["target"])
    tdt = target.dtype
    offset = inputs["offset"]
    seg_w = np.asarray(inputs["seg_w"], np.float64)
    seg_b = np.asarray(inputs["seg_b"], np.float64)
    proj_w1 = np.asarray(inputs["proj_w1"], np.float64)
    proj_w2 = np.asarray(inputs["proj_w2"], np.float64)
    proj_b2 = np.asarray(inputs["proj_b2"], np.float64)
    apd_w1 = np.asarray(inputs["apd_w1"], np.float64)
    apd_w2 = np.asarray(inputs["apd_w2"], np.float64)
    apd_b2 = np.asarray(inputs["apd_b2"], np.float64)
    fp_w1 = np.asarray(inputs["fp_w1"], np.float64)
    bn_g = np.asarray(inputs["bn_g"], np.float64)
    bn_b = np.asarray(inputs["bn_b"], np.float64)
    fp_w2 = np.asarray(inputs["fp_w2"], np.float64)
    fp_b2 = np.asarray(inputs["fp_b2"], np.float64)

    npc = feat.shape[0] // NCORES
    NMT = npc // 512
    nb = feat.shape[0] // B  # points per batch

    tgt = np.asarray(target, np.int32)
    tgall = _prep_common(tgt, npc)

    key = ("A", npc)
    if key not in _CACHE:
        _CACHE[key] = _build_passA(npc)
    ncA = _CACHE[key]

    ident = np.eye(128, dtype=bfnp)
    segwb = np.concatenate([seg_w.T, seg_b[None, :]], 0).astype(bfnp)
    kidxrow = np.tile(np.arange(K, dtype=np.int32), 4)[None, :]

    in_maps = []
    for c in range(NCORES):
        in_maps.append(dict(
            feat=feat[c * npc:(c + 1) * npc],
            tga=tgall[c],
            ident=ident,
            segwb=segwb,
            kidxrow=kidxrow,
        ))
    rA = _RUNNER(ncA, in_maps)

    # -------- host mid glue (float64) --------
    Ms = [np.asarray(rA[c]["outM"], np.float64) for c in range(NCORES)]
    cpb = NCORES // B  # cores per batch
    M_b, sx_b, S1_b, s2_b = [], [], [], []
    segsum = np.zeros((K, C)); counts = np.zeros(K)
    pre_self_num = 0.0
    for c in range(NCORES):
        o = Ms[c]
        segsum += o[0:C, C + 1 + K:C + 1 + 2 * K].T
        counts += o[C, C + 1 + K:C + 1 + 2 * K]
        nll = np.asarray(rA[c]["outnll"], np.float64)
        pre_self_num += nll[:, 0].sum() - nll[:, 1].sum()
    for b in range(B):
        cs = range(b * cpb, (b + 1) * cpb)
        o = sum(Ms[c] for c in cs)
        M_b.append(o[0:C, 0:C])
        sx_b.append(o[C, 0:C])
        S1_b.append(o[0:C, C + 1:C + 1 + K].T)
        s2_b.append(o[C, C + 1:C + 1 + K])
    nvalid = counts.sum()
    pre_self_loss = pre_self_num / max(nvalid, 1.0)

    # BN stats
    sh_b = [sx_b[b] @ fp_w1.T for b in range(B)]
    sh2_b = [np.einsum("ij,jk,ik->i", fp_w1, M_b[b], fp_w1) for b in range(B)]
    mu_b = [sh_b[b] / nb for b in range(B)]
    var_b = [sh2_b[b] / nb - mu_b[b] ** 2 for b in range(B)]
    mu_f = sum(sh_b) / feat.shape[0]
    var_f = sum(sh2_b) / feat.shape[0] - mu_f ** 2

    # protos
    ppn_b = []
    for b in range(B):
        pred_proto = S1_b[b] / (s2_b[b][:, None] + 1e-7)
        pp = _mlp2(np.concatenate([pred_proto, seg_w], 1), proj_w1, proj_w2, proj_b2)
        ppn_b.append(_l2n(pp))
    class_mean = segsum / (counts[:, None] + 1e-4)
    present = counts > 0
    new_proto = np.where(present[:, None], class_mean, seg_w)
    pp_cac = _mlp2(np.concatenate([new_proto, seg_w], 1), apd_w1, apd_w2, apd_b2)
    ppn_cac = _l2n(pp_cac)

    # folds
    def fold(mu, var, ppn):
        s = bn_g / np.sqrt(var + 1e-5)
        if np.any(s <= 0):
            raise RuntimeError("BN scale must be positive for relu fold")
        t = bn_b - mu * s
        tprime = t / s
        W2p = fp_w2 * s[None, :]
        Wl = ppn @ W2p           # [K, C]
        G = W2p.T @ W2p          # [C, C]
        v = 2.0 * (W2p.T @ fp_b2)
        cb = ppn @ fp_b2         # [K]
        return s, tprime, W2p, Wl, G, v, cb

    c0 = float(fp_b2 @ fp_b2)
    has_c0 = abs(c0) > 0
    has_v = bool(np.any(fp_b2 != 0))
    fold_b = [fold(mu_b[b], var_b[b], ppn_b[b]) for b in range(B)]
    fold_f = fold(mu_f, var_f, ppn_cac)
    has_cb = has_v

    keyB = ("B", npc, has_c0, has_v, has_cb)
    if keyB not in _CACHE:
        _CACHE[keyB] = _build_passB(npc, has_c0, has_v, has_cb)
    ncB = _CACHE[keyB]

    in_mapsB = []
    for c in range(NCORES):
        b = c // cpb
        _, tpb, _, Wlb, Gb, vb, cbb = fold_b[b]
        _, tpf, _, Wlf, Gf, vf_, cbf = fold_f
        in_mapsB.append(dict(
            xst=np.asarray(rA[c]["xstore"]),
            tgb=tgall[c],
            w1t=np.ascontiguousarray(fp_w1.T).astype(bfnp),
            gbt=Gb.astype(bfnp),
            gft=Gf.astype(bfnp),
            wrlt=np.ascontiguousarray(Wlb.T).astype(bfnp),
            wcact=np.ascontiguousarray(Wlf.T).astype(bfnp),
            tbv=tpb.astype(np.float32)[:, None],
            tfv=tpf.astype(np.float32)[:, None],
            vbv=vb.astype(np.float32)[:, None],
            vfv=vf_.astype(np.float32)[:, None],
            ones96=np.ones((C, 1), bfnp),
            ident=ident,
            kidxrow=kidxrow,
            cbrow=np.concatenate([cbb, cbf]).astype(np.float32)[None, :],
            c0row=np.full((1, 2), c0, np.float32),
        ))
    rB = _RUNNER(ncB, in_mapsB)

    cols = sum(np.asarray(rB[c]["outcols"], np.float64) for c in range(NCORES))
    num_true = cols[0] / 2.0
    den_true = -cols[1]
    seg_num = -cols[2].sum()
    pre_num = -cols[3].sum()

    cls_loss = num_true / (den_true + 1e-4)
    pf = present.astype(np.float64)
    kl_loss = (cls_loss * pf).sum() / (pf.sum() + 1e-4)
    seg_loss = seg_num / max(nvalid, 1.0)
    pre_loss = pre_num / max(nvalid, 1.0)

    out = seg_loss + pre_loss + pre_self_loss + kl_loss
    return np.float32(out)


# revision 17
# speedup vs baseline: 1.1030x; 1.1030x over previous
"""Trainium2 Bass kernel for nn_CACSegmentor (segment_reduce).

Strategy: shard N=524288 points across 8 cores (65536 each; each core's
slice lies in one point cloud b=core//2). Two SPMD launches:
  pass A: per-point seg logits + softmax P; one fused PE matmul accumulates
          [M=x^T x | S1=P^T x | segment sums | counts | Sigma x]; CE(seg)
          partial sums; stores transposed bf16 feat for pass B.
  host:   tiny [K,C] glue (BN stats from M, proto mlps, weight folds).
  pass B: h=W1 x -> relu_b/relu_f -> z=G relu (norm via quadratic form),
          rl/cac cosine logits, softmax losses, per-class sums via OH matmul.
"""
import sys, os
sys.path.insert(0, "/opt/trn_rl_repo")

import numpy as np
import ml_dtypes
from contextlib import ExitStack

import concourse.bass as bass
import concourse.bacc as bacc
import concourse.tile as tile
from concourse import mybir
from concourse import bass_utils
from concourse.ap import AP

N, C, K, B, NCORES = 524288, 96, 20, 4, 8
NPC = N // NCORES
COS = 15.0
BF = mybir.dt.bfloat16
F32 = mybir.dt.float32
I32 = mybir.dt.int32
bfnp = ml_dtypes.bfloat16
AF = mybir.ActivationFunctionType
OP = mybir.AluOpType
AX = mybir.AxisListType

_CACHE = {}


def _default_runner(nc, in_maps):
    res = bass_utils.run_bass_kernel_spmd(nc, in_maps, list(range(len(in_maps))))
    return res.results


_RUNNER = _default_runner


def _bc(ap, axis, n):
    """Insert a broadcast (0-stride) dim of size n at position axis."""
    return ap.unsqueeze(axis).broadcast_to(
        tuple(ap.shape[:axis]) + (n,) + tuple(ap.shape[axis:]))


# ---------------------------------------------------------------- pass A ----
def _build_passA(npc):
    T = 512
    NMT = npc // T
    nc = bacc.Bacc("TRN2", target_bir_lowering=False, debug=False)
    feat = nc.dram_tensor("feat", [npc, C], F32, kind="ExternalInput").ap()
    tga = nc.dram_tensor("tga", [NMT, 128, 4], I32, kind="ExternalInput").ap()
    ident = nc.dram_tensor("ident", [128, 128], BF, kind="ExternalInput").ap()
    segwb = nc.dram_tensor("segwb", [C + 1, K], BF, kind="ExternalInput").ap()
    kidxrow = nc.dram_tensor("kidxrow", [1, 4 * K], I32, kind="ExternalInput").ap()
    xstore = nc.dram_tensor("xstore", [npc // 512, C, 512], BF, kind="ExternalOutput").ap()
    outM = nc.dram_tensor("outM", [C + 1, C + 1 + 2 * K], F32, kind="ExternalOutput").ap()
    outnll = nc.dram_tensor("outnll", [128, 2], F32, kind="ExternalOutput").ap()

    with tile.TileContext(nc) as tc, ExitStack() as ctx:
        const = ctx.enter_context(tc.tile_pool(name="const", bufs=1))
        identt = const.tile([128, 128], BF)
        nc.sync.dma_start(identt[:], ident)
        segwt = const.tile([C + 1, K], BF)
        nc.sync.dma_start(segwt[:], segwb)
        kid = const.tile([1, 4 * K], I32)
        nc.sync.dma_start(kid[:], kidxrow)
        kidx4 = const.tile([128, 4 * K], I32)
        nc.gpsimd.partition_broadcast(kidx4[:], kid[:])

        acc = ctx.enter_context(tc.tile_pool(name="acc", bufs=1))
        sBb = acc.tile([128, NMT * 4], F32)
        vfb = acc.tile([128, NMT * 4], F32)
        acc2b = acc.tile([128, NMT], F32)
        scrap = acc.tile([128, 4 * K], BF)
        scrap2 = acc.tile([128, NMT * 4], F32)

        psA = ctx.enter_context(tc.tile_pool(name="psA", bufs=3, space="PSUM"))
        psM = ctx.enter_context(tc.tile_pool(name="psM", bufs=1, space="PSUM"))
        bigM = psM.tile([C + 1, C + 1 + 2 * K], F32)

        sb = ctx.enter_context(tc.tile_pool(name="sbA", bufs=4))

        for m in range(NMT):
            xf = sb.tile([128, 4, C], F32, tag="xf")
            for a in range(4):
                nc.sync.dma_start(
                    xf[:, a, :], feat[m * 512 + a * 128: m * 512 + (a + 1) * 128, :])
            tg = sb.tile([128, 4], I32, tag="tg")
            nc.sync.dma_start(tg[:], tga[m])

            # xe free layout: [0:C]=x bf16, [C]=ones, [C+1:C+1+K]=P, [C+1+K:C+1+2K]=OH
            xe = sb.tile([128, 4, C + 1 + 2 * K], BF, tag="xe")
            nc.vector.tensor_copy(xe[:, :, 0:C], xf[:])
            nc.vector.memset(xe[:, :, C:C + 1], 1.0)

            xtp = psA.tile([C + 1, 512], BF, tag="xtp")
            for a in range(4):
                nc.tensor.transpose(
                    xtp[:, a * 128:(a + 1) * 128], xe[:, a, 0:C + 1], identt[:])
            xts = sb.tile([C + 1, 512], BF, tag="xts")
            nc.vector.tensor_copy(xts[:], xtp[:])
            nc.sync.dma_start(xstore[m], xts[0:C, :])

            segp = psA.tile([128, 4, K], F32, tag="segp")
            for a in range(4):
                nc.tensor.matmul(
                    segp[:, a, :], xts[:, a * 128:(a + 1) * 128], segwt[:],
                    start=True, stop=True)

            esb = sb.tile([128, 4, K], F32, tag="esb")
            nc.scalar.activation(esb[:], segp[:], AF.Exp)
            nc.vector.tensor_reduce(
                sBb[:, m * 4:(m + 1) * 4], esb[:], axis=AX.X, op=OP.add)
            rec = sb.tile([128, 4], F32, tag="rec")
            nc.vector.reciprocal(rec[:], sBb[:, m * 4:(m + 1) * 4])
            nc.vector.tensor_tensor(
                xe[:, :, C + 1:C + 1 + K], esb[:], _bc(rec[:], 2, K), op=OP.mult)

            oh = xe[:, :, C + 1 + K:C + 1 + 2 * K]
            nc.vector.tensor_tensor(
                oh, kidx4[:].rearrange("p (a k) -> p a k", a=4),
                _bc(tg[:], 2, K), op=OP.is_equal)
            nc.vector.tensor_reduce(
                vfb[:, m * 4:(m + 1) * 4], oh, axis=AX.X, op=OP.add)
            nc.vector.scalar_tensor_tensor(
                scrap[:].rearrange("p (a k) -> p a k", a=4), oh, 1.0, segp[:],
                op0=OP.mult, op1=OP.mult, accum_out=acc2b[:, m:m + 1])

            for a in range(4):
                nc.tensor.matmul(
                    bigM[:], xe[:, a, 0:C + 1], xe[:, a, 0:C + 1 + 2 * K],
                    start=(m == 0 and a == 0), stop=(m == NMT - 1 and a == 3))

        lnb = acc.tile([128, NMT * 4], F32)
        nc.scalar.activation(lnb[:], sBb[:], AF.Ln)
        accVL = acc.tile([128, 1], F32)
        nc.vector.tensor_tensor(scrap2[:], vfb[:], lnb[:], op=OP.mult)
        nc.vector.tensor_reduce(accVL[:], scrap2[:], axis=AX.X, op=OP.add)
        acc2r = acc.tile([128, 1], F32)
        nc.vector.tensor_reduce(acc2r[:], acc2b[:], axis=AX.X, op=OP.add)
        nc.sync.dma_start(outnll[:, 0:1], accVL[:])
        nc.sync.dma_start(outnll[:, 1:2], acc2r[:])
        bigMs = acc.tile([C + 1, C + 1 + 2 * K], F32)
        nc.vector.tensor_copy(bigMs[:], bigM[:])
        nc.sync.dma_start(outM[:], bigMs[:])

    nc.compile()
    return nc


# ---------------------------------------------------------------- pass B ----
def _build_passB(npc, has_c0, has_v, has_cb):
    T = 512
    NMT = npc // T
    nc = bacc.Bacc("TRN2", target_bir_lowering=False, debug=False)
    xst = nc.dram_tensor("xst", [npc // 512, C, 512], BF, kind="ExternalInput").ap()
    tgb = nc.dram_tensor("tgb", [NMT, 128, 4], I32, kind="ExternalInput").ap()
    w1t = nc.dram_tensor("w1t", [C, C], BF, kind="ExternalInput").ap()
    gbt = nc.dram_tensor("gbt", [C, C], BF, kind="ExternalInput").ap()
    gft = nc.dram_tensor("gft", [C, C], BF, kind="ExternalInput").ap()
    wrlt = nc.dram_tensor("wrlt", [C, K], BF, kind="ExternalInput").ap()
    wcact = nc.dram_tensor("wcact", [C, K], BF, kind="ExternalInput").ap()
    tbv = nc.dram_tensor("tbv", [C, 1], F32, kind="ExternalInput").ap()
    tfv = nc.dram_tensor("tfv", [C, 1], F32, kind="ExternalInput").ap()
    vbv = nc.dram_tensor("vbv", [C, 1], F32, kind="ExternalInput").ap()
    vfv = nc.dram_tensor("vfv", [C, 1], F32, kind="ExternalInput").ap()
    ones96 = nc.dram_tensor("ones96", [C, 1], BF, kind="ExternalInput").ap()
    ident = nc.dram_tensor("ident", [128, 128], BF, kind="ExternalInput").ap()
    kidxrow = nc.dram_tensor("kidxrow", [1, 4 * K], I32, kind="ExternalInput").ap()
    cbrow = nc.dram_tensor("cbrow", [1, 2 * K], F32, kind="ExternalInput").ap()
    c0row = nc.dram_tensor("c0row", [1, 2], F32, kind="ExternalInput").ap()
    outcols = nc.dram_tensor("outcols", [4, K], F32, kind="ExternalOutput").ap()

    LN15 = float(np.log(COS))

    with tile.TileContext(nc) as tc, ExitStack() as ctx:
        const = ctx.enter_context(tc.tile_pool(name="const", bufs=1))

        def cload(apdram, shape, dt):
            t = const.tile(shape, dt, tag=f"c_{apdram.name}")
            nc.sync.dma_start(t[:], apdram)
            return t

        w1tt = cload(w1t, [C, C], BF)
        gbtt = cload(gbt, [C, C], BF)
        gftt = cload(gft, [C, C], BF)
        wrltt = cload(wrlt, [C, K], BF)
        wcactt = cload(wcact, [C, K], BF)
        tbt = cload(tbv, [C, 1], F32)
        tft = cload(tfv, [C, 1], F32)
        vbt = cload(vbv, [C, 1], F32)
        vft = cload(vfv, [C, 1], F32)
        onest = cload(ones96, [C, 1], BF)
        identt = cload(ident, [128, 128], BF)
        kid = cload(kidxrow, [1, 4 * K], I32)
        kidx4 = const.tile([128, 4 * K], I32)
        nc.gpsimd.partition_broadcast(kidx4[:], kid[:])
        cbr = cload(cbrow, [1, 2 * K], F32)
        cbbc = const.tile([128, 2 * K], F32)
        nc.gpsimd.partition_broadcast(cbbc[:], cbr[:])
        c0r = cload(c0row, [1, 2], F32)
        c0bc = const.tile([128, 2], F32)
        nc.gpsimd.partition_broadcast(c0bc[:], c0r[:])
        bias15 = const.tile([128, 1], F32)
        nc.vector.memset(bias15[:], LN15)
        bias4 = const.tile([128, 1], F32)
        nc.vector.memset(bias4[:], 1e-4)

        psH = ctx.enter_context(tc.tile_pool(name="psH", bufs=1, space="PSUM"))
        psB = ctx.enter_context(tc.tile_pool(name="psB", bufs=2, space="PSUM"))
        psU = ctx.enter_context(tc.tile_pool(name="psU", bufs=2, space="PSUM"))
        psC = ctx.enter_context(tc.tile_pool(name="psC", bufs=1, space="PSUM"))
        colacc = psC.tile([4, K], F32)
        sb = ctx.enter_context(tc.tile_pool(name="sbB", bufs=4))

        for m in range(NMT):
            xt = sb.tile([C, 512], BF, tag="xt")
            nc.sync.dma_start(xt[:], xst[m])
            tg = sb.tile([128, 4], I32, tag="tg")
            nc.sync.dma_start(tg[:], tgb[m])

            hp = psH.tile([C, 512], F32, tag="hp")
            nc.tensor.matmul(hp[:], w1tt[:], xt[:], start=True, stop=True)
            rb = sb.tile([C, 512], BF, tag="rb")
            nc.scalar.activation(rb[:], hp[:], AF.Relu, bias=tbt[:])
            rf = sb.tile([C, 512], BF, tag="rf")
            nc.vector.tensor_scalar(
                rf[:], hp[:], tft[:], 0.0, op0=OP.add, op1=OP.max)

            zb = psB.tile([C, 512], F32, tag="z")
            nc.tensor.matmul(zb[:], gbtt[:], rb[:], start=True, stop=True)
            pb = sb.tile([C, 512], BF, tag="pb")
            if has_v:
                nc.vector.scalar_tensor_tensor(
                    pb[:], zb[:], vbt[:], rb[:], op0=OP.add, op1=OP.mult)
            else:
                nc.vector.tensor_tensor(pb[:], zb[:], rb[:], op=OP.mult)
            zf = psB.tile([C, 512], F32, tag="z")
            nc.tensor.matmul(zf[:], gftt[:], rf[:], start=True, stop=True)
            pf = sb.tile([C, 512], BF, tag="pf")
            if has_v:
                nc.vector.scalar_tensor_tensor(
                    pf[:], zf[:], vft[:], rf[:], op0=OP.add, op1=OP.mult)
            else:
                nc.vector.tensor_tensor(pf[:], zf[:], rf[:], op=OP.mult)

            # transpose p_b/p_f subtiles to [pts, C] and reduce -> per-point norms
            s2p = sb.tile([128, 4, 2], F32, tag="s2p")
            for pi, pt in enumerate((pb, pf)):
                ptt = psU.tile([128, 4, C], BF, tag="ptt")
                for a in range(4):
                    nc.tensor.transpose(
                        ptt[:, a, :], pt[:, a * 128:(a + 1) * 128],
                        identt[0:C, 0:C])
                nc.vector.tensor_reduce(
                    s2p[:, :, pi], ptt[:], axis=AX.X, op=OP.add)
            # clamp (and +c0): nsq' = max(nsq + c0, 1e-24)
            if has_c0:
                nc.vector.tensor_tensor(
                    s2p[:], s2p[:], _bc(c0bc[:], 1, 4), op=OP.add)
            nc.vector.tensor_scalar(
                s2p[:], s2p[:], 1e-24, None, op0=OP.max)
            lnn = sb.tile([128, 4, 2], F32, tag="lnn")
            nc.scalar.activation(lnn[:], s2p[:], AF.Ln)
            st = sb.tile([128, 4, 2], F32, tag="st")
            nc.scalar.activation(st[:], lnn[:], AF.Exp, scale=-0.5, bias=bias15[:])

            up = psU.tile([128, 4, 2, K], F32, tag="up")
            for a in range(4):
                nc.tensor.matmul(
                    up[:, a, 0, :], rb[:, a * 128:(a + 1) * 128], wrltt[:],
                    start=True, stop=True)
                nc.tensor.matmul(
                    up[:, a, 1, :], rf[:, a * 128:(a + 1) * 128], wcactt[:],
                    start=True, stop=True)

            rl = sb.tile([128, 4, 2, K], F32, tag="rl")
            if has_cb:
                nc.vector.tensor_tensor(
                    rl[:], up[:],
                    _bc(cbbc[:].rearrange("p (t k) -> p t k", t=2), 1, 4),
                    op=OP.add)
                nc.vector.tensor_tensor(rl[:], rl[:], _bc(st[:], 3, K), op=OP.mult)
            else:
                nc.vector.tensor_tensor(rl[:], up[:], _bc(st[:], 3, K), op=OP.mult)

            e = sb.tile([128, 4, 2, K], F32, tag="e")
            nc.scalar.activation(e[:], rl[:], AF.Exp)
            se = sb.tile([128, 4, 2], F32, tag="se")
            nc.vector.tensor_reduce(se[:], e[:], axis=AX.X, op=OP.add)
            lnse = sb.tile([128, 4, 2], F32, tag="lnse")
            nc.scalar.activation(lnse[:], se[:], AF.Ln)
            rse = sb.tile([128, 4], F32, tag="rse")
            nc.vector.reciprocal(rse[:], se[:, :, 1])

            sm = sb.tile([128, 4, K], F32, tag="sm")
            nc.vector.tensor_tensor(sm[:], e[:, :, 1, :], _bc(rse[:], 2, K),
                                    op=OP.mult)
            lsm0 = sb.tile([128, 4, K], F32, tag="lsm0")
            nc.scalar.activation(lsm0[:], sm[:], AF.Ln, bias=bias4[:])

            oh = sb.tile([128, 4, K], BF, tag="oh")
            nc.vector.tensor_tensor(
                oh[:], kidx4[:].rearrange("p (a k) -> p a k", a=4),
                _bc(tg[:], 2, K), op=OP.is_equal)

            cols = sb.tile([128, 4, 4], F32, tag="cols")
            tmp = sb.tile([128, 4, K], F32, tag="tmp")
            # ent' = sum sm*ln(sm+1e-4)  -> cols[:,:,1]
            nc.vector.tensor_tensor(tmp[:], sm[:], lsm0[:], op=OP.mult)
            nc.vector.tensor_reduce(cols[:, :, 1], tmp[:], axis=AX.X, op=OP.add)
            # lsm_rl = rl_b - lnse_b
            lsmrl = sb.tile([128, 4, K], F32, tag="lsmrl")
            nc.vector.tensor_tensor(
                lsmrl[:], rl[:, :, 0, :], _bc(lnse[:, :, 0], 2, K), op=OP.subtract)
            # A = sum lsm_rl * e_cac
            At = sb.tile([128, 4], F32, tag="At")
            nc.vector.tensor_tensor(tmp[:], lsmrl[:], e[:, :, 1, :], op=OP.mult)
            nc.vector.tensor_reduce(At[:], tmp[:], axis=AX.X, op=OP.add)
            # Bv = sum lsm_rl * OH -> cols[:,:,2]
            nc.vector.tensor_tensor(tmp[:], lsmrl[:], oh[:], op=OP.mult)
            nc.vector.tensor_reduce(cols[:, :, 2], tmp[:], axis=AX.X, op=OP.add)
            # nllc = sum (cac - lnse_cac) * OH -> cols[:,:,3]
            lsmc = sb.tile([128, 4, K], F32, tag="lsmc")
            nc.vector.tensor_tensor(
                lsmc[:], rl[:, :, 1, :], _bc(lnse[:, :, 1], 2, K), op=OP.subtract)
            nc.vector.tensor_tensor(tmp[:], lsmc[:], oh[:], op=OP.mult)
            nc.vector.tensor_reduce(cols[:, :, 3], tmp[:], axis=AX.X, op=OP.add)
            # le'' = (A*rse + Bv) * ent' -> cols[:,:,0]
            lp = sb.tile([128, 4], F32, tag="lp")
            nc.vector.tensor_tensor(lp[:], At[:], rse[:], op=OP.mult)
            nc.vector.tensor_tensor(lp[:], lp[:], cols[:, :, 2], op=OP.add)
            nc.vector.tensor_tensor(cols[:, :, 0], lp[:], cols[:, :, 1], op=OP.mult)

            colsb = sb.tile([128, 4, 4], BF, tag="colsb")
            nc.vector.tensor_copy(colsb[:], cols[:])
            for a in range(4):
                nc.tensor.matmul(
                    colacc[:], colsb[:, a, :], oh[:, a, :],
                    start=(m == 0 and a == 0), stop=(m == NMT - 1 and a == 3))

        colsout = const.tile([4, K], F32)
        nc.vector.tensor_copy(colsout[:], colacc[:])
        nc.sync.dma_start(outcols[:], colsout[:])

    nc.compile()
    return nc


# ------------------------------------------------------------- host glue ----
def _l2n(x, axis=1):
    n = np.linalg.norm(x, axis=axis, keepdims=True)
    return x / np.maximum(n, 1e-12)


def _mlp2(x, w1, w2, b2):
    return np.maximum(x @ w1.T, 0.0) @ w2.T + b2


def _prep_common(target_i32, npc):
    NMT = npc // 512
    tg = target_i32.reshape(NCORES, NMT, 4, 128).transpose(0, 1, 3, 2)
    return np.ascontiguousarray(tg)  # [NCORES, NMT, 128, 4]


def kernel(**inputs):
    feat = np.asarray(inputs["feat"], np.float32)
    target = np.asarray(inputs["target"])
    tdt = target.dtype
    offset = inputs["offset"]
    seg_w = np.asarray(inputs["seg_w"], np.float64)
    seg_b = np.asarray(inputs["seg_b"], np.float64)
    proj_w1 = np.asarray(inputs["proj_w1"], np.float64)
    proj_w2 = np.asarray(inputs["proj_w2"], np.float64)
    proj_b2 = np.asarray(inputs["proj_b2"], np.float64)
    apd_w1 = np.asarray(inputs["apd_w1"], np.float64)
    apd_w2 = np.asarray(inputs["apd_w2"], np.float64)
    apd_b2 = np.asarray(inputs["apd_b2"], np.float64)
    fp_w1 = np.asarray(inputs["fp_w1"], np.float64)
    bn_g = np.asarray(inputs["bn_g"], np.float64)
    bn_b = np.asarray(inputs["bn_b"], np.float64)
    fp_w2 = np.asarray(inputs["fp_w2"], np.float64)
    fp_b2 = np.asarray(inputs["fp_b2"], np.float64)

    npc = feat.shape[0] // NCORES
    NMT = npc // 512
    nb = feat.shape[0] // B  # points per batch

    tgt = np.asarray(target, np.int32)
    tgall = _prep_common(tgt, npc)

    key = ("A", npc)
    if key not in _CACHE:
        _CACHE[key] = _build_passA(npc)
    ncA = _CACHE[key]

    ident = np.eye(128, dtype=bfnp)
    segwb = np.concatenate([seg_w.T, seg_b[None, :]], 0).astype(bfnp)
    kidxrow = np.tile(np.arange(K, dtype=np.int32), 4)[None, :]

    in_maps = []
    for c in range(NCORES):
        in_maps.append(dict(
            feat=feat[c * npc:(c + 1) * npc],
            tga=tgall[c],
            ident=ident,
            segwb=segwb,
            kidxrow=kidxrow,
        ))
    rA = _RUNNER(ncA, in_maps)

    # -------- host mid glue (float64) --------
    Ms = [np.asarray(rA[c]["outM"], np.float64) for c in range(NCORES)]
    cpb = NCORES // B  # cores per batch
    M_b, sx_b, S1_b, s2_b = [], [], [], []
    segsum = np.zeros((K, C)); counts = np.zeros(K)
    pre_self_num = 0.0
    for c in range(NCORES):
        o = Ms[c]
        segsum += o[0:C, C + 1 + K:C + 1 + 2 * K].T
        counts += o[C, C + 1 + K:C + 1 + 2 * K]
        nll = np.asarray(rA[c]["outnll"], np.float64)
        pre_self_num += nll[:, 0].sum() - nll[:, 1].sum()
    for b in range(B):
        cs = range(b * cpb, (b + 1) * cpb)
        o = sum(Ms[c] for c in cs)
        M_b.append(o[0:C, 0:C])
        sx_b.append(o[C, 0:C])
        S1_b.append(o[0:C, C + 1:C + 1 + K].T)
        s2_b.append(o[C, C + 1:C + 1 + K])
    nvalid = counts.sum()
    pre_self_loss = pre_self_num / max(nvalid, 1.0)

    # BN stats
    sh_b = [sx_b[b] @ fp_w1.T for b in range(B)]
    sh2_b = [np.einsum("ij,jk,ik->i", fp_w1, M_b[b], fp_w1) for b in range(B)]
    mu_b = [sh_b[b] / nb for b in range(B)]
    var_b = [sh2_b[b] / nb - mu_b[b] ** 2 for b in range(B)]
    mu_f = sum(sh_b) / feat.shape[0]
    var_f = sum(sh2_b) / feat.shape[0] - mu_f ** 2

    # protos
    ppn_b = []
    for b in range(B):
        pred_proto = S1_b[b] / (s2_b[b][:, None] + 1e-7)
        pp = _mlp2(np.concatenate([pred_proto, seg_w], 1), proj_w1, proj_w2, proj_b2)
        ppn_b.append(_l2n(pp))
    class_mean = segsum / (counts[:, None] + 1e-4)
    present = counts > 0
    new_proto = np.where(present[:, None], class_mean, seg_w)
    pp_cac = _mlp2(np.concatenate([new_proto, seg_w], 1), apd_w1, apd_w2, apd_b2)
    ppn_cac = _l2n(pp_cac)

    # folds
    def fold(mu, var, ppn):
        s = bn_g / np.sqrt(var + 1e-5)
        if np.any(s <= 0):
            raise RuntimeError("BN scale must be positive for relu fold")
        t = bn_b - mu * s
        tprime = t / s
        W2p = fp_w2 * s[None, :]
        Wl = ppn @ W2p           # [K, C]
        G = W2p.T @ W2p          # [C, C]
        v = 2.0 * (W2p.T @ fp_b2)
        cb = ppn @ fp_b2         # [K]
        return s, tprime, W2p, Wl, G, v, cb

    c0 = float(fp_b2 @ fp_b2)
    has_c0 = abs(c0) > 0
    has_v = bool(np.any(fp_b2 != 0))
    fold_b = [fold(mu_b[b], var_b[b], ppn_b[b]) for b in range(B)]
    fold_f = fold(mu_f, var_f, ppn_cac)
    has_cb = has_v

    keyB = ("B", npc, has_c0, has_v, has_cb)
    if keyB not in _CACHE:
        _CACHE[keyB] = _build_passB(npc, has_c0, has_v, has_cb)
    ncB = _CACHE[keyB]

    in_mapsB = []
    for c in range(NCORES):
        b = c // cpb
        _, tpb, _, Wlb, Gb, vb, cbb = fold_b[b]
        _, tpf, _, Wlf, Gf, vf_, cbf = fold_f
        in_mapsB.append(dict(
            xst=np.asarray(rA[c]["xstore"]),
            tgb=tgall[c],
            w1t=np.ascontiguousarray(fp_w1.T).astype(bfnp),
            gbt=Gb.astype(bfnp),
            gft=Gf.astype(bfnp),
            wrlt=np.ascontiguousarray(Wlb.T).astype(bfnp),
            wcact=np.ascontiguousarray(Wlf.T).astype(bfnp),
            tbv=tpb.astype(np.float32)[:, None],
            tfv=tpf.astype(np.float32)[:, None],
            vbv=vb.astype(np.float32)[:, None],
            vfv=vf_.astype(np.float32)[:, None],
            ones96=np.ones((C, 1), bfnp),
            ident=ident,
            kidxrow=kidxrow,
            cbrow=np.concatenate([cbb, cbf]).astype(np.float32)[None, :],
            c0row=np.full((1, 2), c0, np.float32),
        ))
    rB = _RUNNER(ncB, in_mapsB)

    cols = sum(np.asarray(rB[c]["outcols"], np.float64) for c in range(NCORES))
    num_true = cols[0] / 2.0
    den_true = -cols[1]
    seg_num = -cols[2].sum()
    pre_num = -cols[3].sum()

    cls_loss = num_true / (den_true + 1e-4)
    pf = present.astype(np.float64)
    kl_loss = (cls_loss * pf).sum() / (pf.sum() + 1e-4)
    seg_loss = seg_num / max(nvalid, 1.0)
    pre_loss = pre_num / max(nvalid, 1.0)

    out = seg_loss + pre_loss + pre_self_loss + kl_loss
    return np.float32(out)
